# revision 1
# baseline (speedup 1.0000x reference)
"""Haar wavelet (2x2 block) decomposition kernel for 8 Trainium2 NeuronCores.

Input  x: [16, 32, 512, 512] f32
Output  : [16, 128, 256, 256] f32 = concat([pooled, diffH, diffV, diffD], axis=1)

Sharding: pure data parallel over the batch axis — core i handles batches
[2i, 2i+1] (64 images of 512x512 per core).

Per-image dataflow on one core (all fp32):
  load X [128, 2048]  (partition p = image rows 4p..4p+3, one 1 MiB DMA)
  s = E + O, d = E - O          (row butterfly, DVE, FD=1024)
  po = (s_e + s_o) * 0.25       (column butterfly, DVE + ACT scale)
  dv = (s_e - s_o) * 0.5
  dh = (d_e + d_o) * 0.5
  dd =  d_e - d_o
  4 stores of [128, 512] (each a contiguous 256 KiB output image)

The walrus build available here only accepts ONE sync-wait per instruction
(setupSyncWait: "Too many sync wait commands"), while Tile freely attaches
several.  _split_multi_waits() post-processes the serialized BIR, hoisting
all-but-one wait of every instruction onto single-wait NoOps inserted just
before it (same engine, so per-engine program order is preserved).
"""

import functools

import numpy as np
import orjson

import concourse.bass as bass
import concourse.mybir as mybir
from concourse.tile import TileContext

_N_CORES = 8
_B, _C, _H, _W = 16, 32, 512, 512
_BPC = _B // _N_CORES  # batches per core
_IMGS = _BPC * _C  # images per core
_F32 = mybir.dt.float32


def _split_multi_waits(j: dict) -> dict:
    for fn in j["functions"]:
        for blk in fn["blocks"]:
            out = []
            for ins in blk["instructions"]:
                si = ins.get("sync_info")
                waits = (si or {}).get("on_wait") or []
                if len(waits) > 1:
                    for k, w in enumerate(waits[:-1]):
                        out.append(
                            {
                                "debug": ins.get("debug", 0),
                                "engine": ins["engine"],
                                "ins": [],
                                "outs": [],
                                "name": f"{ins['name']}__w{k}",
                                "opcode": "NoOp",
                                "text_hint": "split_wait",
                                "sync_info": {"on_update": [], "on_wait": [w]},
                            }
                        )
                    si["on_wait"] = [waits[-1]]
                out.append(ins)
            blk["instructions"] = out
    return j


if not getattr(bass.Bass.to_json_bytes, "_haar_split_patch", False):
    _orig_to_json_bytes = bass.Bass.to_json_bytes

    def _patched_to_json_bytes(self):
        j = orjson.loads(_orig_to_json_bytes(self))
        _split_multi_waits(j)
        return orjson.dumps(j)

    _patched_to_json_bytes._haar_split_patch = True
    bass.Bass.to_json_bytes = _patched_to_json_bytes


@functools.lru_cache(maxsize=1)
def _build_nc() -> bass.Bass:
    nc = bass.Bass()
    x = nc.dram_tensor("x", [_IMGS, _H, _W], _F32, kind="ExternalInput")
    y = nc.dram_tensor("y", [4 * _IMGS, _H // 2, _W // 2], _F32, kind="ExternalOutput")
    yv = y.rearrange("(b k c) h w -> b c k (h w)", b=_BPC, k=4)

    with TileContext(nc) as tc:
        with tc.tile_pool(name="sbuf", bufs=3) as pool:
            # Two consecutive images (same batch b, channels c, c+1) per
            # iteration, partition-split: partitions 0-63 hold image c,
            # 64-127 hold image c+1, 8 input rows per partition.  Every DMA
            # segment is then a single contiguous run per partition (loads
            # 16 KiB, stores 4 KiB) — HW-measured ~6% faster than the
            # row-interleaved pair layout (432.9 -> 407.4 us/core).  Loads
            # go on the SP HWDGE ring, stores on the ACT ring so both rings
            # drive the SDMA pool concurrently.
            for img0 in range(0, _IMGS, 2):
                X = pool.tile([128, 2 * 4 * _W], _F32, tag="X")
                nc.sync.dma_start(
                    out=X,
                    in_=x[img0 : img0 + 2].rearrange(
                        "i (p a) w -> (i p) (a w)", p=64, a=8
                    ),
                )
                # per partition q: 8 rows = (a = row-pair 0..3, eo = even/odd)
                Xv = X.rearrange("q (a eo w) -> q eo a w", a=4, eo=2)
                s = pool.tile([128, 2 * 2 * _W], _F32, tag="s")
                d = pool.tile([128, 2 * 2 * _W], _F32, tag="d")
                sv = s.rearrange("q (a w) -> q a w", a=4)
                dvv = d.rearrange("q (a w) -> q a w", a=4)
                nc.vector.tensor_add(out=sv, in0=Xv[:, 0], in1=Xv[:, 1])
                nc.vector.tensor_sub(out=dvv, in0=Xv[:, 0], in1=Xv[:, 1])
                # column butterfly: split free dim into (x, v), v = even/odd col
                sr = s.rearrange("q (x v) -> q v x", v=2)
                dr = d.rearrange("q (x v) -> q v x", v=2)
                # all four results live in ONE tile so the pair's outputs ship
                # as a single fused 2 MiB store (32 store completions per core
                # instead of 128; HW-measured 407.4 -> 395.0 us/core)
                O = pool.tile([128, 4 * 2 * _W], _F32, tag="O")
                po = O[:, 0 * 2 * _W : 1 * 2 * _W]
                dh = O[:, 1 * 2 * _W : 2 * 2 * _W]
                dv = O[:, 2 * 2 * _W : 3 * 2 * _W]
                dd = O[:, 3 * 2 * _W : 4 * 2 * _W]
                nc.vector.tensor_add(out=po, in0=sr[:, 0], in1=sr[:, 1])
                nc.vector.tensor_add(out=dh, in0=dr[:, 0], in1=dr[:, 1])
                nc.vector.tensor_sub(out=dv, in0=sr[:, 0], in1=sr[:, 1])
                nc.vector.tensor_sub(out=dd, in0=dr[:, 0], in1=dr[:, 1])
                nc.scalar.mul(po, po, 0.25)
                nc.scalar.mul(dh, dh, 0.5)
                nc.scalar.mul(dv, dv, 0.5)
                b, c0 = divmod(img0, _C)
                nc.scalar.dma_start(
                    out=yv[b, c0 : c0 + 2].rearrange("i k (p aw) -> (i p) k aw", p=64),
                    in_=O.rearrange("q (k aw) -> q k aw", k=4),
                )
    return nc


@functools.lru_cache(maxsize=1)
def _build_runner():
    """Compile once; return a callable shards -> list of per-core outputs.

    Mirrors bass2jax.run_bass_via_pjrt's multi-core path (shard_map over the
    8 axon devices, donated zero output buffers), but keeps the jitted
    function alive so repeated kernel() calls don't recompile the NEFF.
    """
    import jax
    from jax.sharding import Mesh, PartitionSpec, NamedSharding
    from jax.experimental.shard_map import shard_map
    from concourse import bass2jax

    nc = _build_nc()
    partition_name = nc.partition_id_tensor.name if nc.partition_id_tensor else None
    in_names, out_names, out_avals = [], [], []
    for alloc in nc.m.functions[0].allocations:
        if not isinstance(alloc, mybir.MemoryLocationSet):
            continue
        name = alloc.memorylocations[0].name
        if alloc.kind == "ExternalInput":
            if name != partition_name:
                in_names.append(name)
        elif alloc.kind == "ExternalOutput":
            out_names.append(name)
            out_avals.append(
                jax.core.ShapedArray(
                    tuple(alloc.tensor_shape), mybir.dt.np(alloc.dtype)
                )
            )
    n_params = len(in_names)
    n_outs = len(out_names)
    all_in_names = in_names + out_names + ([partition_name] if partition_name else [])

    def _body(*args):
        operands = list(args)
        if partition_name is not None:
            operands.append(bass2jax.partition_id_tensor())
        outs = bass2jax._bass_exec_p.bind(
            *operands,
            out_avals=tuple(out_avals),
            in_names=tuple(all_in_names),
            out_names=tuple(out_names),
            lowering_input_output_aliases=(),
            sim_require_finite=True,
            sim_require_nnan=True,
            nc=nc,
        )
        return tuple(outs)

    bass2jax.install_neuronx_cc_hook()
    devices = jax.devices()[:_N_CORES]
    assert len(devices) == _N_CORES, f"need {_N_CORES} devices, got {len(devices)}"
    mesh = Mesh(np.asarray(devices), ("core",))
    in_specs = (PartitionSpec("core"),) * (n_params + n_outs)
    out_specs = (PartitionSpec("core"),) * n_outs
    sharded = jax.jit(
        shard_map(
            _body, mesh=mesh, in_specs=in_specs, out_specs=out_specs, check_rep=False
        ),
        donate_argnums=tuple(range(n_params, n_params + n_outs)),
        keep_unused=True,
    )
    out_shape = out_avals[0].shape
    zero_shape = (_N_CORES * out_shape[0], *out_shape[1:])
    sh = NamedSharding(mesh, PartitionSpec("core"))
    # allocate + fill the donated output buffer on-device: avoids a 512 MiB
    # host->device transfer of zeros per call
    make_zeros = jax.jit(
        lambda: jax.numpy.zeros(zero_shape, np.float32), out_shardings=sh
    )

    def run(x_global: np.ndarray) -> np.ndarray:
        (out,) = sharded(x_global, make_zeros())
        return np.asarray(out)

    return run


def kernel(x) -> np.ndarray:
    x = np.ascontiguousarray(np.asarray(x), dtype=np.float32)
    assert x.shape == (_B, _C, _H, _W), x.shape
    x_global = x.reshape(_N_CORES * _IMGS, _H, _W)  # view, no copy
    out = _build_runner()(x_global)  # [8*4*_IMGS, 256, 256], core-major
    return out.reshape(_B, 4 * _C, _H // 2, _W // 2)

